# revision 24
# baseline (speedup 1.0000x reference)
"""Multi-head causal attention (B=4, S=2048, D=1024, H=16, HD=64) on 8 TRN2 cores.

Strategy (collective-free):
  - Head-parallel: core i computes heads {2i, 2i+1} for all tokens.
    Host pre-transposes x -> xT [D, B*S], folds the 1/sqrt(HD) scale into Wq,
    converts matmul inputs to bf16.
  - Per core, per batch: q/k/v projections (weights stationary, xT moving;
    Wqkv repacked so one PSUM group yields both heads' q — one bias add per
    group), scores computed transposed [k, q] with the two heads packed via
    PE row tiling (K=64 each), exp on ACT over a strided [128, 2, 512-c0]
    view, PV matmul with stationary [v | 1] so the softmax denominator lands
    in output row 64, then normalize (den copy -> gpsimd broadcast -> fast
    reciprocal -> multiply straight out of PSUM).
  - No AllToAll: each core's 2 heads are a 128-row slice of the D=1024
    contraction in out @ Wo, so each core computes a PARTIAL output
    projection for all 8192 tokens and writes fp16 partials to DRAM.
    The host sums the 8 partials (same device FLOPs, zero collectives).
  - Scheduling: oproj of qi is emitted one slot late so its matmuls are the
    ready PE filler during the next qi's normalization stall; oproj PSUM
    tiles live in the short-lived proj ring so the po accumulators never
    gate the next qi's PV; vtrans is spread across the batch.
"""

import sys

sys.path.insert(0, "/opt/trn_rl_repo")

import numpy as np

import concourse.mybir as mybir
import concourse.tile as tile
from concourse import bacc, bass_utils

FP = mybir.dt.float32
F16 = mybir.dt.float16
BF = mybir.dt.bfloat16
AOP = mybir.AluOpType
AFT = mybir.ActivationFunctionType

B, S, D, H = 4, 2048, 1024, 16
HD = 64
N_CORES = 8
NT = B * S  # 8192 tokens
KD = D // 128  # 8 contraction tiles for the projections


def build_nc():
    nc = bacc.Bacc(None, target_bir_lowering=False, debug=False, num_devices=N_CORES)

    xt = nc.dram_tensor("xt", [16, 128, KD, 512], BF, kind="ExternalInput")
    wqk = nc.dram_tensor("wqk", [128, 2 * KD, 128], BF, kind="ExternalInput")
    wv = nc.dram_tensor("wv", [128, KD, 128], BF, kind="ExternalInput")
    bqk = nc.dram_tensor("bqk", [128, 2], FP, kind="ExternalInput")
    bvb = nc.dram_tensor("bv", [128, 1], FP, kind="ExternalInput")
    wo = nc.dram_tensor("wo", [128, D], BF, kind="ExternalInput")
    maskd = nc.dram_tensor("mask", [128, 2, 128], BF, kind="ExternalInput")
    identd = nc.dram_tensor("ident", [128, 128], BF, kind="ExternalInput")
    out = nc.dram_tensor("out", [NT, D], F16, kind="ExternalOutput")

    with tile.TileContext(nc) as tc:
        with (
            tc.tile_pool(name="const", bufs=1) as const,
            tc.tile_pool(name="xtp", bufs=5) as xtp,
            tc.tile_pool(name="qkv", bufs=2) as qkv,
            tc.tile_pool(name="vnp", bufs=34) as vnp,
            tc.tile_pool(name="esp", bufs=8) as esp,
            tc.tile_pool(name="small", bufs=4) as small,
            tc.tile_pool(name="onp", bufs=4) as onp,
            tc.tile_pool(name="oop", bufs=3) as oop,
            tc.tile_pool(name="ps_mm", bufs=2, space="PSUM") as ps_mm,
            tc.tile_pool(name="ps_s", bufs=2, space="PSUM") as ps_s,
            tc.tile_pool(name="ps_o", bufs=1, space="PSUM") as ps_o,
        ):
            # ---- resident constants ----
            wqk_sb = const.tile([128, 2 * KD, 128], BF, name="wqk_sb")
            nc.scalar.dma_start(wqk_sb[:], wqk[:])
            wv_sb = const.tile([128, KD, 128], BF, name="wv_sb")
            nc.scalar.dma_start(wv_sb[:], wv[:])
            mask_sb = const.tile([128, 2, 128], BF, name="mask_sb")
            nc.scalar.dma_start(mask_sb[:], maskd[:])
            ident_sb = const.tile([128, 128], BF, name="ident_sb")
            nc.scalar.dma_start(ident_sb[:], identd[:])
            bqk_sb = const.tile([128, 2], FP, name="bqk_sb")
            nc.scalar.dma_start(bqk_sb[:], bqk[:])
            bv_sb = const.tile([128, 1], FP, name="bv_sb")
            nc.scalar.dma_start(bv_sb[:], bvb[:])
            wo_sb = const.tile([128, D], BF, name="wo_sb")
            nc.scalar.dma_start(wo_sb[:], wo[:])
            ones2_sb = const.tile([128, 2], BF, name="ones2_sb")
            nc.gpsimd.memset(ones2_sb[:], 1.0)

            qkv_tiles = {}

            def alloc_qkv(b):
                qkv_tiles[b] = (
                    qkv.tile([128, S], BF, name="qT", tag="qT"),
                    qkv.tile([128, S], BF, name="kT", tag="kT"),
                    qkv.tile([128, S], BF, name="vT", tag="vT"),
                )
                vn_tiles[b] = []

            def emit_proj_st(b, st, split_dma=False):
                qT, kT, vT = qkv_tiles[b]
                xt_st = xtp.tile([128, KD, 512], BF, name="xt_st", tag="xt")
                if split_dma:
                    # per-kd chunks so the first matmul group can start early
                    for kd in range(KD):
                        nc.sync.dma_start(xt_st[:, kd, :], xt[4 * b + st, :, kd, :])
                else:
                    nc.sync.dma_start(xt_st[:], xt[4 * b + st])
                xts = [xt_st[:, kd, :] for kd in range(KD)]
                # group 0 -> [q_h0; q_h1], group 1 -> [k_h0; k_h1]
                for g, dst in ((0, qT), (1, kT)):
                    ps = ps_mm.tile([128, 512], FP, name="ps_qk", tag="mm")
                    for kd in range(KD):
                        nc.tensor.matmul(
                            ps[:],
                            lhsT=wqk_sb[:, g * KD + kd, :],
                            rhs=xts[kd],
                            start=(kd == 0),
                            stop=(kd == KD - 1),
                        )
                    nc.vector.tensor_scalar(
                        dst[:, st * 512 : (st + 1) * 512],
                        ps[:],
                        bqk_sb[:, g : g + 1],
                        None,
                        AOP.add,
                    )
                ps = ps_mm.tile([128, 512], FP, name="ps_v", tag="mm")
                for kd in range(KD):
                    nc.tensor.matmul(
                        ps[:],
                        lhsT=wv_sb[:, kd, :],
                        rhs=xts[kd],
                        start=(kd == 0),
                        stop=(kd == KD - 1),
                    )
                nc.vector.tensor_scalar(
                    vT[:, st * 512 : (st + 1) * 512],
                    ps[:],
                    bv_sb[:, 0:1],
                    None,
                    AOP.add,
                )

            vn_tiles = {}

            def emit_vtrans(b, kc0, kc1):
                # vT -> v natural [token, hd] tiles, [h-block | ones] layout
                _, _, vT = qkv_tiles[b]
                for kc in range(kc0, kc1):
                    pst = ps_mm.tile([128, 128], BF, name="ps_t", tag="mm")
                    nc.tensor.transpose(pst[:], vT[:, kc * 128 : (kc + 1) * 128], ident_sb[:])
                    vn = vnp.tile([128, 2, 65], BF, name="vn", tag="vn")
                    nc.vector.tensor_copy(
                        out=vn[:, :, 0:64],
                        in_=pst[:].rearrange("p (h d) -> p h d", h=2),
                    )
                    nc.vector.tensor_copy(out=vn[:, :, 64:65], in_=ones2_sb[:])
                    vn_tiles[b].append(vn)

            def emit_attn_qi(b, qi):
                qT, kT, _ = qkv_tiles[b]
                po = [
                    ps_o.tile([65, 512], FP, name=f"po{h}", tag=f"o{h}")
                    for h in range(2)
                ]
                nki = 4 * (qi + 1)
                for ki in range(nki):
                    j = ki - 4 * qi  # >= 0 on diagonal tiles
                    c0 = 128 * max(j, 0)  # first useful column of this q-tile
                    pss = ps_s.tile([128, 2, 512], FP, name="ps_sc", tag="sc")
                    for h in range(2):
                        nc.tensor.matmul(
                            pss[:, h, c0:512],
                            lhsT=kT[h * 64 : h * 64 + 64, ki * 128 : (ki + 1) * 128],
                            rhs=qT[h * 64 : h * 64 + 64, qi * 512 + c0 : (qi + 1) * 512],
                            start=True,
                            stop=True,
                            tile_position=(h * 64, 0),
                        )
                    es = esp.tile([128, 2, 512], BF, name="es", tag="es")
                    nc.scalar.activation(es[:, :, c0:512], pss[:, :, c0:512], AFT.Exp)
                    if j >= 0:
                        # only the 128-wide boundary block straddles the
                        # diagonal; columns beyond it are fully allowed
                        nc.vector.tensor_tensor(
                            es[:, :, c0 : c0 + 128],
                            es[:, :, c0 : c0 + 128],
                            mask_sb[:],
                            AOP.mult,
                        )
                    for h in range(2):
                        nc.tensor.matmul(
                            po[h][:, c0:512],
                            lhsT=vn_tiles[b][ki][:, h, :],
                            rhs=es[:, h, c0:512],
                            start=(ki == 0),
                            stop=(ki == nki - 1),
                        )
                # normalize into the oproj lhsT tile
                on2 = onp.tile([128, 512], BF, name="on2", tag="on2")
                for h in range(2):
                    den = small.tile([1, 512], FP, name="den", tag="den")
                    nc.vector.tensor_copy(out=den[:], in_=po[h][64:65, :])
                    bc = small.tile([64, 512], FP, name="bc", tag="bc")
                    nc.gpsimd.partition_broadcast(bc[:], den[0:1, :], channels=64)
                    rc = small.tile([64, 512], FP, name="rc", tag="rc")
                    nc.vector.reciprocal_approx_fast(out=rc[:], in_=bc[:])
                    nc.vector.tensor_tensor(
                        on2[h * 64 : (h + 1) * 64, :],
                        po[h][0:64, :],
                        rc[:],
                        AOP.mult,
                    )
                return on2

            def emit_oproj_qi(b, qi, on2):
                # partial output projection (this core's 128-feature slice of
                # the contraction) for this qi's 512 tokens
                tok0 = b * S + qi * 512
                oo = oop.tile([128, 2, 4, 512], F16, name="oo", tag="oo")
                for t in range(4):
                    for nn in range(2):
                        ps = ps_mm.tile([128, 512], FP, name="ps_op", tag="mm")
                        nc.tensor.matmul(
                            ps[:],
                            lhsT=on2[:, t * 128 : (t + 1) * 128],
                            rhs=wo_sb[:, nn * 512 : (nn + 1) * 512],
                            start=True,
                            stop=True,
                        )
                        nc.vector.tensor_copy(out=oo[:, nn, t, :], in_=ps[:])
                for nn in range(2):
                    nc.sync.dma_start(
                        out[tok0 : tok0 + 512, nn * 512 : (nn + 1) * 512].rearrange(
                            "(t p) c -> p t c", p=128
                        ),
                        oo[:, nn],
                    )

            # ---- software-pipelined schedule ----
            # PE warmup on junk data while the first DMAs land (HAM ramp)
            warm = const.tile([128, 640], BF, name="warm")
            nc.gpsimd.memset(warm[:], 0.25)
            wps = ps_mm.tile([128, 512], FP, name="wps", tag="mm")
            for w in range(16):
                nc.tensor.matmul(
                    wps[:],
                    lhsT=warm[:, 0:128],
                    rhs=warm[:, 128:640],
                    start=(w == 0),
                    stop=(w == 15),
                )

            # prologue: batch 0 projections (vtrans spread per-st)
            alloc_qkv(0)
            for st in range(4):
                emit_proj_st(0, st, split_dma=(st == 0))
                emit_vtrans(0, 4 * st, 4 * st + 4)
            # oproj is deferred one qi so its matmuls are the ready PE filler
            # exactly when the next qi's PV stalls on the normalization chain
            pending = None
            for b in range(B):
                last = b == B - 1
                if not last:
                    alloc_qkv(b + 1)
                for qi in range(4):
                    on2 = emit_attn_qi(b, qi)
                    if not last:
                        emit_proj_st(b + 1, qi)
                        emit_vtrans(b + 1, 4 * qi, 4 * qi + 4)
                    if pending is not None:
                        emit_oproj_qi(*pending)
                    pending = (b, qi, on2)
            emit_oproj_qi(*pending)

    nc.finalize()
    return nc


_NC_CACHE = None


def _get_nc():
    global _NC_CACHE
    if _NC_CACHE is None:
        _NC_CACHE = build_nc()
    return _NC_CACHE


def make_in_maps(x, Wqkv, bqkv, Wo):
    import ml_dtypes

    bf16 = ml_dtypes.bfloat16
    scale = HD ** -0.5
    xT = x.reshape(NT, D).T.astype(bf16)  # [D, NT]
    xtn = np.ascontiguousarray(
        xT.reshape(KD, 128, 16, 512).transpose(2, 1, 0, 3)
    )  # [slab, p, kd, t]
    # boundary-block causal mask, duplicated for the two packed heads
    m = (np.arange(128)[None, :] >= np.arange(128)[:, None]).astype(bf16)
    mask2 = np.ascontiguousarray(np.stack([m, m], axis=1))  # [128, 2, 128]
    ident = np.eye(128, dtype=np.float32).astype(bf16)
    in_maps = []
    for c in range(N_CORES):
        h0, h1 = 2 * c, 2 * c + 1
        # group 0 = [q_h0 | q_h1] (scaled), group 1 = [k_h0 | k_h1]
        wqk_c = np.stack(
            [
                np.concatenate(
                    [Wqkv[h0][:, 0:64] * scale, Wqkv[h1][:, 0:64] * scale], axis=1
                ),
                np.concatenate([Wqkv[h0][:, 64:128], Wqkv[h1][:, 64:128]], axis=1),
            ]
        ).astype(bf16)  # [2, D, 128]
        wqk_c = (
            wqk_c.reshape(2, KD, 128, 128).transpose(2, 0, 1, 3).reshape(128, 2 * KD, 128)
        )
        wv_c = np.concatenate(
            [Wqkv[h0][:, 128:192], Wqkv[h1][:, 128:192]], axis=1
        ).astype(bf16)
        wv_c = wv_c.reshape(KD, 128, 128).transpose(1, 0, 2)
        bqk_c = np.stack(
            [
                np.concatenate([bqkv[h0][0:64] * scale, bqkv[h1][0:64] * scale]),
                np.concatenate([bqkv[h0][64:128], bqkv[h1][64:128]]),
            ],
            axis=1,
        ).astype(np.float32)  # [128, 2]
        bv_c = np.concatenate([bqkv[h0][128:192], bqkv[h1][128:192]])[:, None].astype(
            np.float32
        )
        wo_c = np.ascontiguousarray(Wo[128 * c : 128 * (c + 1), :].astype(bf16))
        in_maps.append(
            {
                "xt": xtn,
                "wqk": np.ascontiguousarray(wqk_c),
                "wv": np.ascontiguousarray(wv_c),
                "bqk": np.ascontiguousarray(bqk_c),
                "bv": bv_c,
                "wo": wo_c,
                "mask": mask2,
                "ident": ident,
            }
        )
    return in_maps


def run_cores(in_maps, trace=False, trace_kwargs=None):
    nc = _get_nc()
    kwargs = {}
    if trace:
        kwargs["trace"] = True
        if trace_kwargs:
            kwargs["trace_kwargs"] = trace_kwargs
    return bass_utils.run_bass_kernel_spmd(
        nc, in_maps, core_ids=list(range(N_CORES)), **kwargs
    )


def assemble(results, bo):
    """Sum the per-core partial output projections."""
    full = np.zeros((NT, D), np.float32)
    for c in range(N_CORES):
        full += results[c]["out"].astype(np.float32)
    full += bo[None, :]
    return full.reshape(B, S, D)


def kernel(x, Wqkv, bqkv, Wo, bo):
    x = np.asarray(x, dtype=np.float32)
    Wqkv = np.asarray(Wqkv, dtype=np.float32)
    bqkv = np.asarray(bqkv, dtype=np.float32)
    Wo = np.asarray(Wo, dtype=np.float32)
    bo = np.asarray(bo, dtype=np.float32)

    in_maps = make_in_maps(x, Wqkv, bqkv, Wo)
    res = run_cores(in_maps)
    return assemble(res.results, bo)
